# revision 12
# baseline (speedup 1.0000x reference)
"""Trainium2 Bass kernel for MeshRasterizer (B=2, V=2048, F=512, H=W=128).

Strategy
--------
Every per-(pixel, face) quantity the rasterizer needs for *face selection*
(barycentrics w0/w1/w2 and interpolated depth zp) is an affine function of the
pixel coords (px, py).  So the device work is:

  PE (TensorE): 4 tiny matmuls per image row  [K=3 feats] x [3, F coeffs]
                -> w0, w1, w2, (zp - ZNEAR) as [128 px, 512 faces] PSUM tiles
  DVE/ACT:      validity = min(w0,w1,w2, zp-ZNEAR) >= 0 ; zsort = zp or BIG
                fused tensor_tensor_reduce -> min-z per pixel (= zbuf) and
                argmin face index (iota + BIG*mask, min-reduce)

The device returns only (min z, argmin idx) per pixel.  The host (numpy)
does the O(V+F) projection/coefficient prep, and recomputes the reference
formulas for the *selected* face per pixel (bary, zbuf, signed edge distance)
-- O(B*H*W) work, exactly mirroring reference order of operations.

Sharding: 8 cores; core c handles 32 consecutive image rows of batch
b = c // 4 (data-parallel over B and H, faces replicated), per sharding hint.
"""

import numpy as np

H = 128
Wd = 128
FOCAL = 3.0
ZNEAR = 1e-2
EPS = 1e-8
B, V, F = 2, 2048, 512
NCORES = 8
RPC = (B * H) // NCORES  # image rows per core = 32
BIG = np.float32(1e30)
HIT_THRESH = np.float32(5e29)

_nc_cache = None


def _build_bass():
    """Build the SPMD Bass module (same program for all 8 cores)."""
    global _nc_cache
    if _nc_cache is not None:
        return _nc_cache
    import concourse.mybir as mybir
    from concourse.bacc import Bacc
    from concourse.tile import TileContext

    f32 = mybir.dt.float32
    bf16 = mybir.dt.bfloat16
    Alu = mybir.AluOpType

    # Bacc (not raw Bass): its finalize() runs generate_event_semaphores /
    # move_matmul_waits_to_ldweights, which legalize to walrus's limit of
    # one sync wait per instruction.
    nc = Bacc()
    # feats and coef share one DRAM input (single DMA -> single semaphore so
    # the first matmul carries only one sync wait).  bf16 with 3-way
    # split-precision coefficients packed along K=9: fp32 matmuls lower to a
    # self-loading LW struct that can carry only ONE sync wait in walrus,
    # while bf16 Matmult supports the multi-wait steady state.  Pixel
    # features (odd/128 grid values, 1.0) are exact in bf16 and bf16*bf16
    # products are exact in fp32, so PSUM accumulation matches fp32 accuracy.
    fc = nc.dram_tensor("fc", [9, RPC * 128 + 4 * F], bf16, kind="ExternalInput")
    iotaf = nc.dram_tensor("iotaf", [128, F], f32, kind="ExternalInput")
    outt = nc.dram_tensor("out", [128, 2 * RPC], f32, kind="ExternalOutput")

    with TileContext(nc) as tc:
        with (
            tc.tile_pool(name="const", bufs=1) as cpool,
            tc.tile_pool(name="work", bufs=3) as wpool,
            tc.tile_pool(name="ps", bufs=2, space="PSUM") as ppool,
        ):
            fc_t = cpool.tile([9, RPC * 128 + 4 * F], bf16)
            nc.sync.dma_start(out=fc_t[:], in_=fc[:])
            feats_t = fc_t[:, 0 : RPC * 128]
            coef_t = fc_t[:, RPC * 128 : RPC * 128 + 4 * F]
            iota_t = cpool.tile([128, F], f32)
            nc.sync.dma_start(out=iota_t[:], in_=iotaf[:])
            stage = cpool.tile([128, 2 * RPC], f32)

            for r in range(RPC):
                lhsT = feats_t[:, r * 128 : (r + 1) * 128]

                # all 4 affine maps into ONE PSUM tile (4 banks):
                # [w0 | w1 | w2 | zq], each [128, F]
                pq = ppool.tile([128, 4 * F], f32, tag="pq")
                for q in range(4):
                    nc.tensor.matmul(
                        pq[:, q * F : (q + 1) * F], lhsT,
                        coef_t[:, q * F : (q + 1) * F],
                    )

                # e = min over the 4 quantities (each op touches ONE psum bank)
                s0 = wpool.tile([128, F], f32, tag="s0")
                nc.scalar.copy(s0[:], pq[:, 0:F])
                mn = wpool.tile([128, F], f32, tag="mn")
                nc.vector.tensor_tensor(mn[:], s0[:], pq[:, F : 2 * F], Alu.min)
                nc.vector.tensor_tensor(mn[:], mn[:], pq[:, 2 * F : 3 * F], Alu.min)
                e = wpool.tile([128, F], f32, tag="e")
                nc.vector.tensor_tensor(e[:], mn[:], pq[:, 3 * F : 4 * F], Alu.min)

                # bigm = BIG where invalid else 0
                bigm = wpool.tile([128, F], f32, tag="bigm")
                nc.vector.tensor_scalar(
                    out=bigm[:], in0=e[:], scalar1=0.0, scalar2=float(BIG),
                    op0=Alu.is_lt, op1=Alu.mult,
                )

                # zsort = max(zq, bigm): zq for valid faces (zq>0), BIG else.
                # (max-combining, not add/ttr: those hit a HW exec-unit fault
                # in this environment; see transcript bisection)
                zsort = wpool.tile([128, F], f32, tag="zsort")
                nc.vector.tensor_tensor(
                    zsort[:], bigm[:], pq[:, 3 * F : 4 * F], Alu.max
                )
                nc.vector.tensor_reduce(
                    stage[:, 2 * r : 2 * r + 1], zsort[:],
                    axis=mybir.AxisListType.X, op=Alu.min,
                )

                # notm = BIG where zsort != minz else 0
                notm = wpool.tile([128, F], f32, tag="notm")
                nc.vector.tensor_scalar(
                    out=notm[:], in0=zsort[:],
                    scalar1=stage[:, 2 * r : 2 * r + 1], scalar2=float(BIG),
                    op0=Alu.not_equal, op1=Alu.mult,
                )

                # idx = reduce_min(max(iota, notm)): first index achieving minz
                idxv = wpool.tile([128, F], f32, tag="idxv")
                nc.vector.tensor_tensor(idxv[:], notm[:], iota_t[:], Alu.max)
                nc.vector.tensor_reduce(
                    stage[:, 2 * r + 1 : 2 * r + 2], idxv[:],
                    axis=mybir.AxisListType.X, op=Alu.min,
                )

            nc.sync.dma_start(out=outt[:], in_=stage[:])

    nc.finalize()
    _nc_cache = nc
    return nc


def _ndc_grid():
    return (1.0 - (2.0 * np.arange(H, dtype=np.float32) + 1.0) / H).astype(np.float32)


def _project(verts_world, R, T):
    v_view = np.einsum("bvj,bjk->bvk", verts_world, R).astype(np.float32) + T[:, None, :]
    z = v_view[..., 2:3]
    xy = np.float32(FOCAL) * v_view[..., :2] / z
    return np.concatenate([xy, z], axis=-1).astype(np.float32)  # [B,V,3]


def _face_coeffs(verts_ndc, faces):
    """Per-face affine coefficients for w0,w1,w2,(zp-ZNEAR) over (px,py,1)."""
    tri = verts_ndc[:, faces]  # [B,F,3,3]
    x0, y0, z0 = tri[:, :, 0, 0], tri[:, :, 0, 1], tri[:, :, 0, 2]
    x1, y1, z1 = tri[:, :, 1, 0], tri[:, :, 1, 1], tri[:, :, 1, 2]
    x2, y2, z2 = tri[:, :, 2, 0], tri[:, :, 2, 1], tri[:, :, 2, 2]
    area = (x1 - x0) * (y2 - y0) - (y1 - y0) * (x2 - x0)
    denom = np.where(np.abs(area) > EPS, area, np.float32(EPS)).astype(np.float32)

    a0 = -(y2 - y1) / denom
    b0 = (x2 - x1) / denom
    c0 = ((y2 - y1) * x1 - (x2 - x1) * y1) / denom
    a1 = -(y0 - y2) / denom
    b1 = (x0 - x2) / denom
    c1 = ((y0 - y2) * x2 - (x0 - x2) * y2) / denom
    a2 = -(a0 + a1)
    b2 = -(b0 + b1)
    c2 = np.float32(1.0) - c0 - c1
    az = a0 * z0 + a1 * z1 + a2 * z2
    bz = b0 * z0 + b1 * z1 + b2 * z2
    cz = c0 * z0 + c1 * z1 + c2 * z2 - np.float32(ZNEAR)

    # degenerate faces can never be valid: force w0 hugely negative
    bad = np.abs(area) <= EPS
    c0 = np.where(bad, np.float32(-BIG), c0)

    coef = np.zeros((B, 3, 4 * F), dtype=np.float32)
    for q, (aq, bq, cq) in enumerate(((a0, b0, c0), (a1, b1, c1), (a2, b2, c2), (az, bz, cz))):
        coef[:, 0, q * F : (q + 1) * F] = aq
        coef[:, 1, q * F : (q + 1) * F] = bq
        coef[:, 2, q * F : (q + 1) * F] = cq
    return coef, tri, area, denom


def _run_device(in_maps, trace=False):
    from concourse.bass_utils import run_bass_kernel_spmd

    nc = _build_bass()
    return run_bass_kernel_spmd(nc, in_maps, core_ids=list(range(NCORES)), trace=trace)


def _make_in_maps(verts_world, faces, R, T):
    verts_ndc = _project(verts_world, R, T)
    coef, tri, area, denom = _face_coeffs(verts_ndc, faces)
    s = _ndc_grid()

    iota_c = np.ascontiguousarray(
        np.broadcast_to(np.arange(F, dtype=np.float32), (128, F))
    )
    import ml_dtypes

    bf16 = ml_dtypes.bfloat16
    # 3-way split of the f32 coefficients into bf16 hi/mid/lo terms
    chi = coef.astype(bf16)
    r1 = coef - chi.astype(np.float32)
    cmid = r1.astype(bf16)
    r2 = r1 - cmid.astype(np.float32)
    clo = r2.astype(bf16)

    in_maps = []
    for c in range(NCORES):
        g0 = c * RPC
        b = g0 // H
        fc_c = np.zeros((9, RPC * 128 + 4 * F), dtype=bf16)
        for r in range(RPC):
            h = (g0 + r) % H
            for k in range(3):  # features exact in bf16, replicated per split
                fc_c[3 * k + 0, r * 128 : (r + 1) * 128] = s
                fc_c[3 * k + 1, r * 128 : (r + 1) * 128] = bf16(float(s[h]))
                fc_c[3 * k + 2, r * 128 : (r + 1) * 128] = 1.0
        fc_c[0:3, RPC * 128 :] = chi[b]
        fc_c[3:6, RPC * 128 :] = cmid[b]
        fc_c[6:9, RPC * 128 :] = clo[b]
        in_maps.append({
            "fc": fc_c,
            "iotaf": iota_c,
        })
    return in_maps, (tri, area, denom, s)


def _seg_d2(px, py, ax, ay, bx, by):
    abx, aby = bx - ax, by - ay
    apx, apy = px - ax, py - ay
    t = np.clip((apx * abx + apy * aby) / (abx * abx + aby * aby + np.float32(EPS)), 0.0, 1.0).astype(np.float32)
    dx, dy = apx - t * abx, apy - t * aby
    return dx * dx + dy * dy


def _assemble(results, host_data):
    tri, area, denom, s = host_data

    minz = np.zeros((B * H, 128), dtype=np.float32)
    idxf = np.zeros((B * H, 128), dtype=np.float32)
    for c in range(NCORES):
        out = results[c]["out"]  # [128 (w), 2*RPC]
        g0 = c * RPC
        for r in range(RPC):
            minz[g0 + r, :] = out[:, 2 * r]
            idxf[g0 + r, :] = out[:, 2 * r + 1]
    minz = minz.reshape(B, H, Wd)
    idxf = idxf.reshape(B, H, Wd)

    hit = minz < HIT_THRESH
    idx = np.where(hit, idxf, 0.0).astype(np.int32)  # [B,H,W]

    # gather selected-face vertices and recompute reference formulas per pixel
    px = s[None, None, :]            # varies over W
    py = s[None, :, None]            # varies over H
    bidx = np.arange(B)[:, None, None]
    tsel = tri[bidx, idx]            # [B,H,W,3,3]
    x0, y0, z0 = tsel[..., 0, 0], tsel[..., 0, 1], tsel[..., 0, 2]
    x1, y1, z1 = tsel[..., 1, 0], tsel[..., 1, 1], tsel[..., 1, 2]
    x2, y2, z2 = tsel[..., 2, 0], tsel[..., 2, 1], tsel[..., 2, 2]
    den = denom[bidx, idx]

    w0 = ((x2 - x1) * (py - y1) - (y2 - y1) * (px - x1)) / den
    w1 = ((x0 - x2) * (py - y2) - (y0 - y2) * (px - x2)) / den
    w2 = np.float32(1.0) - w0 - w1
    zp = w0 * z0 + w1 * z1 + w2 * z2
    inside = (w0 >= 0) & (w1 >= 0) & (w2 >= 0)
    d2 = np.minimum(
        _seg_d2(px, py, x0, y0, x1, y1),
        np.minimum(_seg_d2(px, py, x1, y1, x2, y2), _seg_d2(px, py, x2, y2, x0, y0)),
    )
    dists = np.where(inside, -d2, d2).astype(np.float32)

    neg1 = np.float32(-1.0)
    zbuf = np.where(hit, zp, neg1).astype(np.float32)
    bary = np.stack([w0, w1, w2], axis=-1)
    bary = np.where(hit[..., None], bary, neg1).astype(np.float32)
    dsel = np.where(hit, dists, neg1).astype(np.float32)
    p2f = np.where(hit, idx, -1).astype(np.int32)

    return (
        p2f[..., None],
        zbuf[..., None],
        bary[:, :, :, None, :],
        dsel[..., None],
    )


def kernel(**inputs):
    verts_world = np.asarray(inputs["verts_world"], dtype=np.float32)
    faces = np.asarray(inputs["faces"])
    R = np.asarray(inputs["R"], dtype=np.float32)
    T = np.asarray(inputs["T"], dtype=np.float32)

    in_maps, host_data = _make_in_maps(verts_world, faces, R, T)
    res = _run_device(in_maps)
    return _assemble(res.results, host_data)
